# revision 2
# baseline (speedup 1.0000x reference)
"""Trainium2 Bass kernel for nn_AMM_66829691126233 (sparse_attention) — v3.

Computation (see reference):
  theta = concat([fm_source*mask_source*0.01, rel_pos_source], ch).reshape(3, 392, 4096)
  phi   = concat([fm_reference*mask_ref*0.01, rel_pos_ref], ch).reshape(3, 392, 4096)
  scores = theta^T @ phi                      (3, 4096, 4096)
  attn = softmax(scores*200, -1) * (scores != 0)   [mask is a numerical no-op]
  g3 = (w_lambda . fm_reference) * mask_ref   (3, 4096)
  b3 = (w_beta   . fm_reference) * mask_ref
  gamma = sum_b attn[b] @ g3[b];  beta likewise   (4096,)
  out = fm_source * (1 + gamma) + beta        (1, 256, 64, 64)

Sharding: core k owns queries [512k, 512(k+1)), all keys. No collectives.

v3 architecture (vs v2's query-layout flash): two phases.
  Phase 1 (query layout): bf16 hi*hi approx scores (chunks A=128ch, B=8ch tail)
    per [128q, 1024k] psum tile; per-row max via one tensor_reduce from psum.
    bound[b,q] = max_approx + 0.06 (approx err <= ~0.02 score units; window
    needs [max-0.44, max+0.25] for fp32 exp). -bound lands as an extra matmul
    channel: row 72 of T5 (theta) against a ones row 72 in P5 (phi).
  Phase 2 (key layout): per (part, key-tile kt) scores_T [128key, 512q] =
    sum over 6(+bound) chunks of matmul(lhsT=phi_chunk[:,kt], rhs=theta_chunk).
    exp on ACT with scale=200, NO bias (bound folded), out bf16.
    z/ng/nb via PE: matmul(Z[32b:32b+3,:], lhsT=V_kt[128,3], rhs=e) with
    V = [ones, w_g, w_b] per key-tile, accumulated over all 32 kt into one
    col-grouped psum bank. No DVE weighted sums, no flash rescale epilogue.

Precision scheme (same as v2): rel channels as bf16 hi/lo packed pairs;
score = hi.hi + hi.lo + lo.hi over chunks c2..c5; fm chunks plain bf16:
  c2[0:128] = (hi[0:128],  hi[0:128])
  c3[0:96]  = (hi[0:96],   lo[0:96]);  c3[96:104] = (hi[128:136], hi[128:136])
  c4[0:64]  = (lo[0:64],   hi[0:64]);  c4[64:104] = (hi[96:136],  lo[96:136])
  c5[0:72]  = (lo[64:136], hi[64:136]);  c5[72] = (-bound_q, ones)
(theta-side listed first; in phase 2 phi is lhsT and theta is rhs.)
"""

import sys

if "/opt/trn_rl_repo" not in sys.path:
    sys.path.insert(0, "/opt/trn_rl_repo")

import numpy as np

H = W = 64
HW = H * W          # 4096
C_FM = 256
C_REL = 136
NP = 3
NCORES = 8
S = HW // NCORES    # 512 query pixels per core
NT = S // 128       # 4 query row-tiles (phase 1)
MH = HW // 2        # 2048: keys per prep half
KT = HW // 128      # 32 key-tiles (phase 2)
KTH = KT // 2       # 16 key-tiles per half
KSP = HW // 512     # 8 phase-1 key spans of 512
MARGIN = 0.06

_cache = {}


def _build(debug=False):
    import concourse.bass as bass
    import concourse.tile as tile
    from concourse import bacc, mybir
    from concourse.masks import make_identity

    f32 = mybir.dt.float32
    f16 = mybir.dt.float16
    bf16 = mybir.dt.bfloat16
    AF = mybir.ActivationFunctionType
    ALU = mybir.AluOpType

    nc = bacc.Bacc(None, target_bir_lowering=False, debug=debug)

    fm_src_sl = nc.declare_dram_parameter("fm_src_sl", [C_FM, S], f32, isOutput=False)
    mask_src_sl = nc.declare_dram_parameter("mask_src_sl", [NP, S], f32, isOutput=False)
    rel_src_sl = nc.declare_dram_parameter("rel_src_sl", [NP, C_REL, S], f32, isOutput=False)
    fm_ref = nc.declare_dram_parameter("fm_ref", [C_FM, HW], f32, isOutput=False)
    mask_ref = nc.declare_dram_parameter("mask_ref", [NP, HW], f32, isOutput=False)
    rel_ref = nc.declare_dram_parameter("rel_ref", [NP, C_REL, HW], f32, isOutput=False)
    w_lambda = nc.declare_dram_parameter("w_lambda", [1, C_FM], f32, isOutput=False)
    w_beta = nc.declare_dram_parameter("w_beta", [1, C_FM], f32, isOutput=False)
    out_ext = nc.declare_dram_parameter("out", [C_FM, S], f32, isOutput=True)

    og_dram = nc.dram_tensor("og_scratch", [2, HW], bf16)
    mkb_dram = nc.dram_tensor("mkb_scratch", [NP, HW], bf16)
    bnd_dram = nc.dram_tensor("bnd_scratch", [NP, 4, 128], f16)
    gb_dram = nc.dram_tensor("gb_scratch", [2, S], f32)

    with tile.TileContext(nc) as tc:
        with (
            tc.tile_pool(name="static", bufs=1) as st,
            tc.tile_pool(name="early_p", bufs=4) as ep,   # per-part early (theta)
            tc.tile_pool(name="early_h", bufs=4) as eh,   # per-(part,half) early (phi)
            tc.tile_pool(name="late_p", bufs=2) as lp,    # per-part late (theta)
            tc.tile_pool(name="late_h", bufs=2) as lh,    # per-(part,half) late (phi)
            tc.tile_pool(name="work", bufs=3) as wk,      # e tiles
            tc.tile_pool(name="stats", bufs=1) as sp,
            tc.tile_pool(name="psum", bufs=2, space="PSUM") as pm,   # [128,1024] x2 = 4 banks
            tc.tile_pool(name="psum1", bufs=2, space="PSUM") as p1,  # ph1 [128,512] x2 = 2 banks
            tc.tile_pool(name="psumz", bufs=1, space="PSUM") as pz,  # Z accumulator, 1 bank
            tc.tile_pool(name="psumt", bufs=1, space="PSUM") as pt,  # transposes etc, 1 bank
            tc.tile_pool(name="dram", bufs=2, space="DRAM") as dp,
        ):
            # ---------------- phase 0: weights, fm prep ----------------
            # w_lambda / w_beta as [128ch, 2] columns per channel-chunk
            wv = []
            for c in range(2):
                t = st.tile([128, 2], bf16, name=f"wv{c}")
                nc.gpsimd.dma_start(out=t[:, 0:1], in_=w_lambda[0:1, c * 128:(c + 1) * 128])
                nc.gpsimd.dma_start(out=t[:, 1:2], in_=w_beta[0:1, c * 128:(c + 1) * 128])
                wv.append(t)

            fmr_bf = []
            for c in range(2):
                t = st.tile([128, HW], bf16, name=f"fmr_bf{c}")
                nc.gpsimd.dma_start(out=t[:], in_=fm_ref[c * 128:(c + 1) * 128, :])
                fmr_bf.append(t)
            fms_raw = []
            fms_bf = []
            for c in range(2):
                t = st.tile([128, S], f32, name=f"fms_raw{c}")
                nc.sync.dma_start(out=t[:], in_=fm_src_sl[c * 128:(c + 1) * 128, :])
                fms_raw.append(t)
                tb = st.tile([128, S], bf16, name=f"fms_bf{c}")
                nc.gpsimd.dma_start(out=tb[:], in_=fm_src_sl[c * 128:(c + 1) * 128, :])
                fms_bf.append(tb)

            ident = st.tile([128, 128], f32)
            make_identity(nc, ident[:])

            # old_gamma/old_beta rows: og[v, k] = sum_c w[v][c] * fm_ref[c, k]
            # (emitted interleaved into phase-1(b0) to fill PE stall slots)
            og_sb = st.tile([2, HW], bf16, name="og_sb")

            def og_group(i):
                ps_w = pm.tile([128, 1024], f32, tag="ps", name=f"ps_og{i}")
                for k in range(2):
                    col = slice(k * 512, (k + 1) * 512)
                    src = slice(i * 1024 + k * 512, i * 1024 + (k + 1) * 512)
                    for c in range(2):
                        nc.tensor.matmul(
                            ps_w[0:2, col], wv[c][:], fmr_bf[c][:, src],
                            start=(c == 0), stop=(c == 1),
                        )
                nc.scalar.copy(og_sb[:, i * 1024:(i + 1) * 1024], ps_w[0:2, :])

            def og_finish():
                nc.sync.dma_start(out=og_dram[:], in_=og_sb[:])
                # scale fm chunks by 0.01 in place (raw bf16 dead after og mms)
                for c in range(2):
                    nc.vector.tensor_scalar_mul(fmr_bf[c][:], fmr_bf[c][:], 0.01)
                    nc.vector.tensor_scalar_mul(fms_bf[c][:], fms_bf[c][:], 0.01)

            fmr01, fms01 = fmr_bf, fms_bf

            # per-part V matrices [128key, 32kt, 3] = (ones, w_g, w_b) columns
            # (built after phase-1(b0) so the DVE queue isn't blocked on the og chain)
            VB = []

            def build_vb(b):
                # ogc[p, kt] = og[v, kt*128+p] via xbar transpose DMA ([32,128]->[128,32])
                ogc = [st.tile([128, KT], bf16, name=f"ogc{b}{v}") for v in range(2)]
                for v in range(2):
                    nc.sync.dma_start(
                        out=ogc[v][:],
                        in_=og_dram[v].rearrange("(kt p) -> kt p", p=128),
                        transpose=True,
                    )
                # mask is f32 in dram; transpose DMA is 16-bit only, so cast to
                # bf16 via an SBUF+DRAM bounce first
                mks = st.tile([1, HW], bf16, name=f"mks{b}")
                nc.gpsimd.dma_start(out=mks[:], in_=mask_ref[b:b + 1, :])
                nc.sync.dma_start(out=mkb_dram[b], in_=mks[:])
                mkc = st.tile([128, KT], bf16, name=f"mkc{b}")
                nc.sync.dma_start(
                    out=mkc[:],
                    in_=mkb_dram[b].rearrange("(kt p) -> kt p", p=128),
                    transpose=True,
                )
                V = st.tile([128, KT, 3], bf16, name=f"V{b}")
                nc.vector.memset(V[:, :, 0], 1.0)
                nc.vector.tensor_tensor(out=V[:, :, 1], in0=ogc[0][:], in1=mkc[:], op=ALU.mult)
                nc.vector.tensor_tensor(out=V[:, :, 2], in0=ogc[1][:], in1=mkc[:], op=ALU.mult)
                VB.append(V)

            # selector matrices built as [1, N] patterns (aligned) then DMA-reshaped
            # selz: z_b -> psA rows b and 3+b; selw: ng_b -> row b, nb_b -> row 3+b;
            # ones2: psC row0 = sum_b W6[b] (gamma), row1 = sum_b W6[3+b] (beta)
            pat = st.tile([1, 67 * 6 + 6 * 2 + 6 * 2], f32, name="selpat")
            nc.vector.memset(pat[:], 0.0)
            OFW = 67 * 6
            OF2 = OFW + 6 * 2
            for b in range(NP):
                nc.vector.memset(pat[0:1, (32 * b + 1) * 6 + b:(32 * b + 1) * 6 + b + 1], 1.0)
                nc.vector.memset(pat[0:1, (32 * b + 2) * 6 + 3 + b:(32 * b + 2) * 6 + 4 + b], 1.0)
                nc.vector.memset(pat[0:1, OFW + b * 2:OFW + b * 2 + 1], 1.0)
                nc.vector.memset(pat[0:1, OFW + (3 + b) * 2 + 1:OFW + (3 + b) * 2 + 2], 1.0)
            selz = st.tile([67, 6], f32, name="selz")
            nc.vector.memset(selz[:], 0.0)
            for b in range(NP):
                nc.vector.memset(selz[32 * b:32 * b + 1, b:b + 1], 1.0)
                nc.vector.memset(selz[32 * b:32 * b + 1, 3 + b:4 + b], 1.0)
            selw = st.tile([67, 6], f32, name="selw")
            nc.sync.dma_start(out=selw[:], in_=pat[0:1, 0:OFW])
            ones2 = st.tile([6, 2], f32, name="ones2")
            nc.sync.dma_start(out=ones2[:], in_=pat[0:1, OFW:OF2])

            # phase-1 stats: per part [128, NT*KSP] raw max
            mx = [sp.tile([128, NT * KSP], f32, tag=f"mx{b}", name=f"mx{b}") for b in range(NP)]

            EP = {}   # per-part early theta
            EH = {}   # per-(part,half) early phi
            TP = {}   # per-part late theta
            PH = {}   # per-(part,half) late phi

            def early_part(b):
                d = {}
                T2x = ep.tile([128, S + 32], bf16, tag="T2x", name=f"T2x{b}")
                nc.gpsimd.dma_start(out=T2x[:, 0:S], in_=rel_src_sl[b, 0:128, :])
                nc.gpsimd.dma_start(out=T2x[:, S:S + 32], in_=rel_src_sl[b, 128:C_REL, :])
                Ttail = ep.tile([8, S], bf16, tag="Ttail", name=f"Ttail{b}")
                nc.gpsimd.dma_start(out=Ttail[:], in_=rel_src_sl[b, 128:C_REL, :])
                d.update(T2x=T2x, Ttail=Ttail)
                EP[b] = d

            def early_half(b, h):
                hsl = slice(h * MH, (h + 1) * MH)
                P2x = eh.tile([128, MH], bf16, tag="P2x", name=f"P2x{b}{h}")
                nc.gpsimd.dma_start(out=P2x[:], in_=rel_ref[b, 0:128, hsl])
                Ptail = eh.tile([8, MH], bf16, tag="Ptail", name=f"Ptail{b}{h}")
                nc.gpsimd.dma_start(out=Ptail[:], in_=rel_ref[b, 128:C_REL, hsl])
                EH[(b, h)] = dict(P2x=P2x, Ptail=Ptail)

            def late_part(b):
                """theta-side lo/cross tensors + fm for part b."""
                d = {}
                T2x = EP[b]["T2x"]
                mrep = lp.tile([128, HW], bf16, tag="mask_rep", name=f"mask_rep{b}")
                nc.gpsimd.dma_start(out=mrep[:], in_=mask_ref[b:b + 1, :].to_broadcast([128, HW]))
                d["mask"] = mrep
                msrc = lp.tile([128, S], bf16, tag="msrc_rep", name=f"msrc_rep{b}")
                nc.gpsimd.dma_start(out=msrc[:], in_=mask_src_sl[b:b + 1, :].to_broadcast([128, S]))

                raw_t = lp.tile([128, S + 32], f32, tag="raw_tax", bufs=1, name=f"raw_tax{b}")
                nc.sync.dma_start(out=raw_t[:, 0:S], in_=rel_src_sl[b, 0:128, :])
                nc.sync.dma_start(out=raw_t[:, S:S + 32], in_=rel_src_sl[b, 128:C_REL, :])

                T4 = lp.tile([128, S], bf16, tag="T4", name=f"T4_{b}")
                nc.vector.tensor_tensor(out=T4[:, :], in0=raw_t[:, 0:S], in1=T2x[:, 0:S],
                                        op=ALU.subtract)
                ttailf = lp.tile([128, 32], bf16, tag="ttailf", bufs=1, name=f"ttailf{b}")
                nc.vector.tensor_tensor(out=ttailf[:], in0=raw_t[:, S:S + 32],
                                        in1=T2x[:, S:S + 32], op=ALU.subtract)
                ttaild = dp.tile([8, S], bf16, tag="ttaild", name=f"ttaild{b}")
                nc.sync.dma_start(out=ttaild[:], in_=ttailf[:])
                T5 = lp.tile([128, S], f16, tag="T5", name=f"T5_{b}")
                nc.gpsimd.dma_start(out=T5[0:64, :], in_=T4[64:128, :])
                nc.sync.dma_start(out=T4[64:96, :], in_=T2x[96:128, 0:S])
                nc.gpsimd.dma_start(out=T4[96:104, :], in_=rel_src_sl[b, 128:C_REL, :])
                nc.vector.memset(T5[64:96, :], 0.0)
                nc.gpsimd.dma_start(out=T5[64:72, :], in_=ttaild[:])
                # T5 row 96 (-bound) written later, after phase-1 max for part b
                T3 = lp.tile([128, S], bf16, tag="T3", name=f"T3_{b}")
                nc.sync.dma_start(out=T3[0:96, :], in_=T2x[0:96, 0:S])
                nc.gpsimd.dma_start(out=T3[96:104, :], in_=rel_src_sl[b, 128:C_REL, :])

                th_fm = []
                for c in range(2):
                    t = lp.tile([128, S], bf16, tag=f"th_fm{c}", name=f"th_fm{b}{c}")
                    nc.vector.tensor_tensor(out=t[:], in0=fms01[c][:], in1=msrc[:], op=ALU.mult)
                    th_fm.append(t)
                d.update(T3=T3, T4=T4, T5=T5, th_fm=th_fm)
                TP[b] = d

            def late_half(b, h):
                """phi-side lo/cross tensors + fm for part b, half h."""
                hsl = slice(h * MH, (h + 1) * MH)
                P2x = EH[(b, h)]["P2x"]
                mrep = TP[b]["mask"]
                raw = lh.tile([128, MH + 128], f32, tag="raw_ax", bufs=1, name=f"raw{b}{h}")
                nc.sync.dma_start(out=raw[:, 0:MH], in_=rel_ref[b, 0:128, hsl])
                nc.sync.dma_start(out=raw[:, MH:MH + 128], in_=rel_ref[b, 128:C_REL, hsl])
                P2t = lh.tile([128, 128], bf16, tag="P2t", name=f"P2t{b}{h}")
                nc.gpsimd.dma_start(out=P2t[:], in_=rel_ref[b, 128:C_REL, hsl])

                P3 = lh.tile([128, MH], bf16, tag="P3", name=f"P3_{b}{h}")
                nc.vector.tensor_tensor(out=P3[:, :], in0=raw[:, 0:MH], in1=P2x[:], op=ALU.subtract)
                tailf = lh.tile([128, 128], bf16, tag="tailf", bufs=1, name=f"tailf{b}{h}")
                nc.vector.tensor_tensor(out=tailf[:], in0=raw[:, MH:MH + 128], in1=P2t[:],
                                        op=ALU.subtract)
                taild = dp.tile([8, MH], bf16, tag="taild", name=f"taild{b}{h}")
                nc.sync.dma_start(out=taild[:], in_=tailf[:])
                P4 = lh.tile([128, MH], bf16, tag="P4", name=f"P4_{b}{h}")
                nc.sync.dma_start(out=P4[64:96, :], in_=P3[96:128, :])
                nc.gpsimd.dma_start(out=P3[96:104, :], in_=rel_ref[b, 128:C_REL, hsl])
                nc.sync.dma_start(out=P4[96:104, :], in_=taild[:])
                nc.sync.dma_start(out=P4[0:64, :], in_=P2x[0:64, :])
                P5 = lh.tile([128, MH], f16, tag="P5", name=f"P5_{b}{h}")
                nc.gpsimd.dma_start(out=P5[0:64, :], in_=P2x[64:128, :])
                # rows 64:96 zeroed first (aligned base 64); tail overwrites 64:72,
                # rows 72:96 stay zero, ones row at aligned base 96 (bound channel)
                nc.vector.memset(P5[64:96, :], 0.0)
                nc.gpsimd.dma_start(out=P5[64:72, :], in_=rel_ref[b, 128:C_REL, hsl])
                nc.vector.memset(P5[96:97, :], 1.0)

                ph_fm = []
                for c in range(2):
                    t = lh.tile([128, MH], bf16, tag=f"ph_fm{c}", name=f"ph_fm{b}{h}{c}")
                    nc.vector.tensor_tensor(out=t[:], in0=fmr01[c][:, hsl], in1=mrep[:, hsl],
                                            op=ALU.mult)
                    ph_fm.append(t)
                PH[(b, h)] = dict(P3=P3, P4=P4, P5=P5, ph_fm=ph_fm)

            # ---------------- phase 1: approx scores + per-query max ----------------
            def ph1_tile(b, nt, ks):
                """hi*hi approx scores for queries [nt*128,...), keys [ks*512,...)."""
                t = EP[b]
                h = ks // 4
                e = EH[(b, h)]
                nsl = slice(nt * 128, (nt + 1) * 128)
                ksl = slice((ks % 4) * 512, (ks % 4 + 1) * 512)
                ps = p1.tile([128, 512], f32, tag="p1", name=f"p1_{b}{nt}{ks}")
                nc.tensor.matmul(ps[:], t["T2x"][0:128, nsl], e["P2x"][:, ksl],
                                 start=True, stop=False)
                nc.tensor.matmul(ps[:], t["Ttail"][0:8, nsl], e["Ptail"][0:8, ksl],
                                 start=False, stop=True)
                nc.vector.tensor_reduce(out=mx[b][:, nt * KSP + ks:nt * KSP + ks + 1],
                                        in_=ps[:], axis=mybir.AxisListType.X, op=ALU.max)

            def ph1_bound(b):
                """combine per-span maxes -> -bound row into bnd_dram[b]."""
                mxq = sp.tile([128, NT], f32, tag=f"mxq{b}", name=f"mxq{b}")
                nc.vector.tensor_reduce(out=mxq[:],
                                        in_=mx[b][:].rearrange("p (t k) -> p t k", k=KSP),
                                        axis=mybir.AxisListType.X, op=ALU.max)
                negb = sp.tile([128, NT], f32, tag=f"negb{b}", name=f"negb{b}")
                nc.vector.tensor_scalar(out=negb[:], in0=mxq[:], scalar1=MARGIN, scalar2=-1.0,
                                        op0=ALU.add, op1=ALU.mult)
                psT = pt.tile([128, 512], f32, tag="pt", name=f"ptr{b}")
                nc.tensor.transpose(psT[0:NT, 0:128], negb[:], ident[:])
                bnd_sb = sp.tile([NT, 128], f16, tag=f"bnd{b}", name=f"bnd{b}")
                nc.scalar.copy(bnd_sb[:], psT[0:NT, 0:128])
                nc.sync.dma_start(out=bnd_dram[b], in_=bnd_sb[:])

            def t5_bound_row(b):
                nc.sync.dma_start(out=TP[b]["T5"][96:97, 0:S], in_=bnd_dram[b])

            # ---------------- phase 2: full scores in key layout ----------------
            Zt = pz.tile([128, 512], f32, name="Zt")

            def ph2_group(b, g):
                """2 key-tiles kt=2g,2g+1: scores_T -> exp -> Z accum."""
                t = TP[b]
                te = EP[b]
                ps = pm.tile([128, 1024], f32, tag="ps", name=f"p2_{b}{g}")
                for j in range(2):
                    kt = 2 * g + j
                    h = kt // KTH
                    ktl = slice((kt % KTH) * 128, (kt % KTH + 1) * 128)
                    e = EH[(b, h)]
                    p = PH[(b, h)]
                    col = slice(j * 512, (j + 1) * 512)
                    chunks = [
                        (e["P2x"][:, ktl], te["T2x"][0:128, 0:S]),
                        (p["P3"][0:104, ktl], t["T3"][0:104, :]),
                        (p["P4"][0:104, ktl], t["T4"][0:104, :]),
                        (p["P5"][0:97, ktl], t["T5"][0:97, :]),
                        (p["ph_fm"][0][:, ktl], t["th_fm"][0][:]),
                        (p["ph_fm"][1][:, ktl], t["th_fm"][1][:]),
                    ]
                    nch = len(chunks)
                    for ci, (lhsT, rhs) in enumerate(chunks):
                        nc.tensor.matmul(ps[:, col], lhsT, rhs,
                                         start=(ci == 0), stop=(ci == nch - 1))
                e_t = wk.tile([128, 1024], bf16, tag="e", name=f"e{b}{g}")
                nc.scalar.activation(out=e_t[:], in_=ps[:], func=AF.Exp, scale=200.0)
                return (b, g, e_t)

            def z_mms(ctx):
                """Z-reduction matmuls for a group whose exp was emitted one
                group ago — keeps the PE FIFO from blocking on the ACT exp."""
                b, g, e_t = ctx
                for j in range(2):
                    kt = 2 * g + j
                    nc.tensor.matmul(
                        Zt[32 * b:32 * b + 3, :],
                        VB[b][:, kt, :],
                        e_t[:, j * 512:(j + 1) * 512],
                        start=(kt == 0), stop=(kt == KT - 1),
                        tile_position=(0, 32 * b),
                        skip_group_check=True,
                    )

            # ---------------- schedule ----------------
            early_part(0)
            early_half(0, 0)
            early_half(0, 1)
            early_part(1)
            early_half(1, 0)
            early_half(1, 1)
            # ph1(b0) with og matmul groups interleaved into its PE stall slots
            for i in range(NT * KSP):
                ph1_tile(0, i // KSP, i % KSP)
                if i % 8 == 7:
                    og_group(i // 8)
            og_finish()
            for b in range(NP):
                build_vb(b)
            ph1_bound(0)
            late_part(0)
            late_half(0, 0)
            late_half(0, 1)
            t5_bound_row(0)

            # ph2(b) with next part's prep + phase 1 interleaved across groups.
            # late prep first (its DVE work must precede the MAX reduces in the
            # DVE queue so ph2(nb) isn't gated on it), but late_half(nb,1) after
            # the ph1 tiles (its pool buffer recycles (b-1,1), read until ~g7).
            for b in range(NP):
                nb = b + 1
                items = []
                if nb < NP:
                    for i in range(4):
                        items.append(lambda i=i, nb=nb: ph1_tile(nb, i // KSP, i % KSP))
                    items.append(lambda nb=nb: late_part(nb))
                    items.append(lambda nb=nb: late_half(nb, 0))
                    for i in range(4, NT * KSP):
                        items.append(lambda i=i, nb=nb: ph1_tile(nb, i // KSP, i % KSP))
                    items.append(lambda nb=nb: ph1_bound(nb))
                    items.append(lambda nb=nb: t5_bound_row(nb))
                    items.append(lambda nb=nb: late_half(nb, 1))
                    if nb + 1 < NP:
                        items.append(lambda nb=nb: early_part(nb + 1))
                        items.append(lambda nb=nb: early_half(nb + 1, 0))
                        items.append(lambda nb=nb: early_half(nb + 1, 1))
                done = 0
                pending = None
                for g in range(KTH):
                    ctx = ph2_group(b, g)
                    if pending is not None:
                        z_mms(pending)
                    pending = ctx
                    want = (g + 1) * len(items) // KTH if items else 0
                    while done < want:
                        items[done]()
                        done += 1
                z_mms(pending)

            # ---------------- epilogue ----------------
            # Zt rows: 32b+0 = z_b, 32b+1 = ng_b, 32b+2 = nb_b
            # selector matmuls gather rows into aligned psum tiles
            # copy only the 9 live Zt rows; everything else zeroed (uninitialized
            # PSUM can hold NaN bit patterns and 0*NaN would poison the selector
            # matmul contraction)
            Zs = st.tile([128, 512], f32, name="Zs")
            nc.vector.memset(Zs[0:67, :], 0.0)
            for b in range(NP):
                nc.scalar.copy(Zs[32 * b:32 * b + 3, :], Zt[32 * b:32 * b + 3, :])
            psA = pt.tile([128, 512], f32, tag="pt", name="psA")
            nc.tensor.matmul(psA[0:6, :], selz[0:67, 0:6], Zs[0:67, :], start=True, stop=True)
            rz6 = st.tile([6, 512], f32, name="rz6")
            nc.vector.reciprocal(rz6[:], psA[0:6, :])
            psB = pt.tile([128, 512], f32, tag="pt", name="psB")
            nc.tensor.matmul(psB[0:6, :], selw[0:67, 0:6], Zs[0:67, :], start=True, stop=True)
            W6 = st.tile([6, 512], f32, name="W6")
            nc.vector.tensor_tensor(out=W6[0:6, :], in0=psB[0:6, :], in1=rz6[:], op=ALU.mult)
            psC = pt.tile([128, 512], f32, tag="pt", name="psC")
            nc.tensor.matmul(psC[0:2, :], ones2[0:6, :], W6[0:6, :], start=True, stop=True)
            nc.vector.tensor_scalar_add(psC[0:1, :], psC[0:1, :], 1.0)
            gb2 = st.tile([2, 512], f32, name="gb2")
            nc.scalar.copy(gb2[:], psC[0:2, :])
            nc.sync.dma_start(out=gb_dram[:], in_=gb2[:])

            g1_rep = st.tile([128, S], f32)
            nc.sync.dma_start(out=g1_rep[:], in_=gb_dram[0:1, :].to_broadcast([128, S]))
            b_rep = st.tile([128, S], f32)
            nc.sync.dma_start(out=b_rep[:], in_=gb_dram[1:2, :].to_broadcast([128, S]))

            for c in range(2):
                o_t = wk.tile([128, S], f32, tag="o", name=f"o{c}", bufs=2)
                nc.vector.tensor_tensor(out=o_t[:], in0=fms_raw[c][:], in1=g1_rep[:], op=ALU.mult)
                nc.vector.tensor_tensor(out=o_t[:], in0=o_t[:], in1=b_rep[:], op=ALU.add)
                nc.sync.dma_start(out=out_ext[c * 128:(c + 1) * 128, :], in_=o_t[:])

    nc.compile()
    return nc


def kernel(fm_source, fm_reference, mask_source, mask_ref,
           rel_pos_source, rel_pos_ref, w_lambda, w_beta):
    from concourse.bass_utils import run_bass_kernel_spmd

    if "nc" not in _cache:
        _cache["nc"] = _build()
    nc = _cache["nc"]

    fm_src = np.ascontiguousarray(np.asarray(fm_source, np.float32).reshape(C_FM, HW))
    fm_refm = np.ascontiguousarray(np.asarray(fm_reference, np.float32).reshape(C_FM, HW))
    m_src = np.ascontiguousarray(np.asarray(mask_source, np.float32).reshape(NP, HW))
    m_ref = np.ascontiguousarray(np.asarray(mask_ref, np.float32).reshape(NP, HW))
    r_src = np.ascontiguousarray(np.asarray(rel_pos_source, np.float32).reshape(NP, C_REL, HW))
    r_ref = np.ascontiguousarray(np.asarray(rel_pos_ref, np.float32).reshape(NP, C_REL, HW))
    w_l = np.ascontiguousarray(np.asarray(w_lambda, np.float32).reshape(1, C_FM))
    w_b = np.ascontiguousarray(np.asarray(w_beta, np.float32).reshape(1, C_FM))

    in_maps = []
    for k in range(NCORES):
        sl = slice(k * S, (k + 1) * S)
        in_maps.append({
            "fm_src_sl": np.ascontiguousarray(fm_src[:, sl]),
            "mask_src_sl": np.ascontiguousarray(m_src[:, sl]),
            "rel_src_sl": np.ascontiguousarray(r_src[:, :, sl]),
            "fm_ref": fm_refm,
            "mask_ref": m_ref,
            "rel_ref": r_ref,
            "w_lambda": w_l,
            "w_beta": w_b,
        })

    res = run_bass_kernel_spmd(nc, in_maps, list(range(NCORES)))
    _cache["last_result"] = res

    out = np.concatenate([res.results[k]["out"] for k in range(NCORES)], axis=1)
    return out.reshape(1, C_FM, H, W).astype(np.float32)


# revision 7
# speedup vs baseline: 1.2359x; 1.2359x over previous
"""Trainium2 Bass kernel for nn_AMM_66829691126233 (sparse_attention).

Computation (see reference):
  theta = concat([fm_source*mask_source*0.01, rel_pos_source], ch).reshape(3, 392, 4096)
  phi   = concat([fm_reference*mask_ref*0.01, rel_pos_ref], ch).reshape(3, 392, 4096)
  scores = theta^T @ phi                      (3, 4096, 4096)
  attn = softmax(scores*200, -1) * (scores != 0)
  g3 = (w_lambda . fm_reference) * mask_ref   (3, 4096)
  b3 = (w_beta   . fm_reference) * mask_ref
  gamma = sum_b attn[b] @ g3[b];  beta likewise   (4096,)
  out = fm_source * (1 + gamma) + beta        (1, 256, 64, 64)

Sharding: embarrassingly parallel over query rows; core k owns pixels
[512k, 512(k+1)). Flash-style fused softmax+weighted sums; the 3x4096x4096
score matrix never leaves PSUM. No collectives.

The (scores != 0) mask is a numerical no-op (exact zeros sit >=7000 logits
below the row max after the x200 scale; their softmax weight underflows to 0).

v2 layout: the phi-side hi/lo-split tensors are built per (part, key-half)
with bufs=2 tile rotation, and prep for step s+1 is emitted before the
matmul tiles of step s, so DMA/DVE prep overlaps the PE main loop and the
PE never idles long enough for HAM to re-throttle. The two weighted sums
are split by column between DVE and GPSIMD (K_SG).

Precision scheme (unchanged from v1): rel channels as bf16 hi/lo packed
pairs; score contribution = hi.hi + hi.lo + lo.hi over chunks c2..c5:
  c2[0:128] = (th_hi[0:128],  ph_hi[0:128])
  c3[0:96]  = (th_hi[0:96],   ph_lo[0:96]);  c3[96:104] = (th_hi[128:136], ph_hi[128:136])
  c4[0:64]  = (th_lo[0:64],   ph_hi[0:64]);  c4[64:104] = (th_hi[96:136],  ph_lo[96:136])
  c5[0:72]  = (th_lo[64:136], ph_hi[64:136])
fm chunks (x0.01, masked) ride as plain bf16.
"""

import sys

if "/opt/trn_rl_repo" not in sys.path:
    sys.path.insert(0, "/opt/trn_rl_repo")

import os as _os

import numpy as np

H = W = 64
HW = H * W          # 4096
C_FM = 256
C_REL = 136
NP = 3
NCORES = 8
S = HW // NCORES    # 512 query pixels per core
NT = S // 128       # 4 query row-tiles per part
MH = 2048           # key-dim span per psum tile (half of HW)
NSUB = MH // 512    # 512-wide psum banks per span
NH = HW // MH       # key spans (halves) per row-tile

TTR2 = _os.environ.get("K_TTR2", "0") == "1"  # tensor_tensor_reduce (BROKEN on hw)
GSUB = _os.environ.get("K_GSUB", "0") == "1"  # hi/lo subtracts on gpsimd (slower: port contention)
GWGB = _os.environ.get("K_GWGB", "0") == "1"  # w_g/w_b mask mults on gpsimd (slower)
BYP = _os.environ.get("K_BYP", "0") == "1"    # stt op0=bypass variant

_cache = {}


def _build(debug=False):
    import concourse.bass as bass
    import concourse.tile as tile
    from concourse import bacc, mybir
    from concourse.masks import make_identity

    f32 = mybir.dt.float32
    f16 = mybir.dt.float16
    bf16 = mybir.dt.bfloat16
    AF = mybir.ActivationFunctionType
    ALU = mybir.AluOpType

    nc = bacc.Bacc(None, target_bir_lowering=False, debug=debug)

    fm_src_sl = nc.declare_dram_parameter("fm_src_sl", [C_FM, S], f32, isOutput=False)
    mask_src_sl = nc.declare_dram_parameter("mask_src_sl", [NP, S], f32, isOutput=False)
    rel_src_sl = nc.declare_dram_parameter("rel_src_sl", [NP, C_REL, S], f32, isOutput=False)
    fm_ref = nc.declare_dram_parameter("fm_ref", [C_FM, HW], f32, isOutput=False)
    mask_ref = nc.declare_dram_parameter("mask_ref", [NP, HW], f32, isOutput=False)
    rel_ref = nc.declare_dram_parameter("rel_ref", [NP, C_REL, HW], f32, isOutput=False)
    w_lambda = nc.declare_dram_parameter("w_lambda", [1, C_FM], f32, isOutput=False)
    w_beta = nc.declare_dram_parameter("w_beta", [1, C_FM], f32, isOutput=False)
    out_ext = nc.declare_dram_parameter("out", [C_FM, S], f32, isOutput=True)

    gb_dram = nc.dram_tensor("gb_scratch", [8, 128], f32)

    with tile.TileContext(nc) as tc:
        with (
            tc.tile_pool(name="static", bufs=1) as st,
            tc.tile_pool(name="perpart", bufs=2) as pp,
            tc.tile_pool(name="perhalf", bufs=2) as hh,
            tc.tile_pool(name="work", bufs=1) as wk,
            tc.tile_pool(name="stats", bufs=1) as sp,
            tc.tile_pool(name="psum", bufs=2, space="PSUM") as pm,
            tc.tile_pool(name="dram", bufs=2, space="DRAM") as dp,
        ):
            # ---------------- phase 0: weights, fm prep ----------------
            wlam_row = st.tile([1, C_FM], f32)
            nc.sync.dma_start(out=wlam_row[:], in_=w_lambda[0:1, :])
            wbeta_row = st.tile([1, C_FM], f32)
            nc.sync.dma_start(out=wbeta_row[:], in_=w_beta[0:1, :])
            ones1 = st.tile([1, 128], f32)
            nc.vector.memset(ones1[:], 1.0)

            fmr_bf = []
            for c in range(2):
                t = st.tile([128, HW], bf16, name=f"fmr_bf{c}")
                nc.gpsimd.dma_start(out=t[:], in_=fm_ref[c * 128:(c + 1) * 128, :])
                fmr_bf.append(t)
            fms_raw = []
            fms_bf = []
            for c in range(2):
                t = st.tile([128, S], f32, name=f"fms_raw{c}")
                nc.sync.dma_start(out=t[:], in_=fm_src_sl[c * 128:(c + 1) * 128, :])
                fms_raw.append(t)
                tb = st.tile([128, S], bf16, name=f"fms_bf{c}")
                nc.gpsimd.dma_start(out=tb[:], in_=fm_src_sl[c * 128:(c + 1) * 128, :])
                fms_bf.append(tb)

            # replicate w_lambda / w_beta chunks across partitions via K=1 matmul
            wrep_bf = []  # [wl0, wl1, wb0, wb1]
            for q, (row, c) in enumerate([(wlam_row, 0), (wlam_row, 1), (wbeta_row, 0), (wbeta_row, 1)]):
                ps_w = pm.tile([128, MH], f32, tag="ps", name=f"ps_w{q}")
                nc.tensor.matmul(
                    ps_w[:, 0:128],
                    row[0:1, c * 128:(c + 1) * 128],
                    ones1[0:1, :],
                    start=True, stop=True,
                )
                t = st.tile([128, 128], bf16, name=f"wrep{q}")
                nc.scalar.copy(t[:], ps_w[:, 0:128])
                wrep_bf.append(t)

            # old_gamma / old_beta replicated on all 128 partitions: [128, HW] bf16
            old_rep = []
            for vi in range(2):
                dst = st.tile([128, HW], bf16, name=f"old_rep{vi}")
                for hhh in range(NH):
                    pg = pm.tile([128, MH], f32, tag="ps", name=f"ps_old{vi}{hhh}")
                    for k in range(NSUB):
                        col = slice(k * 512, (k + 1) * 512)
                        src = slice(hhh * MH + k * 512, hhh * MH + (k + 1) * 512)
                        for c in range(2):
                            nc.tensor.matmul(
                                pg[:, col],
                                wrep_bf[2 * vi + c][:],
                                fmr_bf[c][:, src],
                                start=(c == 0), stop=(c == 1),
                            )
                    nc.scalar.copy(dst[:, hhh * MH:(hhh + 1) * MH], pg[:])
                old_rep.append(dst)

            # scale fm chunks by 0.01 in place (raw bf16 copies are dead after
            # the old_gamma/old_beta matmuls above)
            for c in range(2):
                nc.vector.tensor_scalar_mul(fmr_bf[c][:], fmr_bf[c][:], 0.01)
                nc.vector.tensor_scalar_mul(fms_bf[c][:], fms_bf[c][:], 0.01)
            fmr01, fms01 = fmr_bf, fms_bf

            ident = st.tile([128, 128], f32)
            make_identity(nc, ident[:])

            # persistent per-part stats (col = nt*NH + h)
            stats = []
            for b in range(NP):
                stats.append({
                    k: sp.tile([128, NT * NH], f32, tag=f"{k}{b}", name=f"{k}{b}")
                    for k in ("nm", "z", "ng", "nb")
                })

            # shared junk output for the accumulating weighted sums
            junk_d = wk.tile([128, MH], bf16, tag="junk_d", name="junk_d")

            # ---------------- prep emitters ----------------
            TP = {}  # per-part theta-side tensors
            PH = {}  # per-(part, half) phi-side tensors

            def part_prep(b):
                """theta-side (query) tensors + masks for part b."""
                d = {}
                mrep = pp.tile([128, HW], bf16, tag="mask_rep", name=f"mask_rep{b}")
                nc.gpsimd.dma_start(out=mrep[:], in_=mask_ref[b:b + 1, :].to_broadcast([128, HW]))
                d["mask"] = mrep
                msrc = pp.tile([128, S], bf16, tag="msrc_rep", name=f"msrc_rep{b}")
                nc.gpsimd.dma_start(out=msrc[:], in_=mask_src_sl[b:b + 1, :].to_broadcast([128, S]))

                raw_t = pp.tile([128, S + 32], f32, tag="raw_tax", bufs=1, name=f"raw_tax{b}")
                nc.sync.dma_start(out=raw_t[:, 0:S], in_=rel_src_sl[b, 0:128, :])
                nc.sync.dma_start(out=raw_t[:, S:S + 32], in_=rel_src_sl[b, 128:C_REL, :])
                T2x = pp.tile([128, S + 32], bf16, tag="T2x", name=f"T2x{b}")
                nc.gpsimd.dma_start(out=T2x[:, 0:S], in_=rel_src_sl[b, 0:128, :])
                nc.gpsimd.dma_start(out=T2x[:, S:S + 32], in_=rel_src_sl[b, 128:C_REL, :])

                sub_eng = nc.gpsimd if GSUB else nc.vector
                T4 = pp.tile([128, S], bf16, tag="T4", name=f"T4_{b}")
                sub_eng.tensor_tensor(out=T4[:, :], in0=raw_t[:, 0:S], in1=T2x[:, 0:S],
                                      op=ALU.subtract)
                ttailf = pp.tile([128, 32], bf16, tag="ttailf", bufs=1, name=f"ttailf{b}")
                sub_eng.tensor_tensor(out=ttailf[:], in0=raw_t[:, S:S + 32],
                                      in1=T2x[:, S:S + 32], op=ALU.subtract)
                ttaild = dp.tile([8, S], bf16, tag="ttaild", name=f"ttaild{b}")
                nc.sync.dma_start(out=ttaild[:], in_=ttailf[:])
                T5 = pp.tile([128, S], bf16, tag="T5", name=f"T5_{b}")
                nc.sync.dma_start(out=T5[0:64, :], in_=T4[64:128, :])
                nc.sync.dma_start(out=T4[64:96, :], in_=T2x[96:128, 0:S])
                nc.gpsimd.dma_start(out=T4[96:104, :], in_=rel_src_sl[b, 128:C_REL, :])
                nc.sync.dma_start(out=T5[64:72, :], in_=ttaild[:])
                T3 = pp.tile([128, S], bf16, tag="T3", name=f"T3_{b}")
                nc.sync.dma_start(out=T3[0:96, :], in_=T2x[0:96, 0:S])
                nc.gpsimd.dma_start(out=T3[96:104, :], in_=rel_src_sl[b, 128:C_REL, :])

                th_fm = []
                for c in range(2):
                    t = pp.tile([128, S], bf16, tag=f"th_fm{c}", name=f"th_fm{b}{c}")
                    nc.vector.tensor_tensor(out=t[:], in0=fms01[c][:], in1=msrc[:], op=ALU.mult)
                    th_fm.append(t)
                d.update(T2x=T2x, T3=T3, T4=T4, T5=T5, th_fm=th_fm)
                TP[b] = d

            def half_prep(b, h):
                """phi-side (key) tensors for part b, key-half h."""
                hsl = slice(h * MH, (h + 1) * MH)
                mrep = TP[b]["mask"]
                raw = hh.tile([128, MH + 128], f32, tag="raw_ax", bufs=1, name=f"raw{b}{h}")
                nc.sync.dma_start(out=raw[:, 0:MH], in_=rel_ref[b, 0:128, hsl])
                nc.sync.dma_start(out=raw[:, MH:MH + 128], in_=rel_ref[b, 128:C_REL, hsl])
                P2x = hh.tile([128, MH], bf16, tag="P2x", name=f"P2x{b}{h}")
                nc.gpsimd.dma_start(out=P2x[:], in_=rel_ref[b, 0:128, hsl])
                P2t = hh.tile([128, 128], bf16, tag="P2t", name=f"P2t{b}{h}")
                nc.gpsimd.dma_start(out=P2t[:], in_=rel_ref[b, 128:C_REL, hsl])

                sub_eng = nc.gpsimd if GSUB else nc.vector
                P3 = hh.tile([128, MH], bf16, tag="P3", name=f"P3_{b}{h}")
                sub_eng.tensor_tensor(out=P3[:, :], in0=raw[:, 0:MH], in1=P2x[:], op=ALU.subtract)
                tailf = hh.tile([128, 128], bf16, tag="tailf", bufs=1, name=f"tailf{b}{h}")
                sub_eng.tensor_tensor(out=tailf[:], in0=raw[:, MH:MH + 128], in1=P2t[:],
                                      op=ALU.subtract)
                taild = dp.tile([8, MH], bf16, tag="taild", name=f"taild{b}{h}")
                nc.sync.dma_start(out=taild[:], in_=tailf[:])
                P4 = hh.tile([128, MH], bf16, tag="P4", name=f"P4_{b}{h}")
                nc.sync.dma_start(out=P4[64:96, :], in_=P3[96:128, :])
                nc.gpsimd.dma_start(out=P3[96:104, :], in_=rel_ref[b, 128:C_REL, hsl])
                nc.sync.dma_start(out=P4[96:104, :], in_=taild[:])
                nc.sync.dma_start(out=P4[0:64, :], in_=P2x[0:64, :])
                P5 = hh.tile([128, MH], bf16, tag="P5", name=f"P5_{b}{h}")
                nc.sync.dma_start(out=P5[0:64, :], in_=P2x[64:128, :])
                nc.gpsimd.dma_start(out=P5[64:72, :], in_=rel_ref[b, 128:C_REL, hsl])

                ph_fm = []
                for c in range(2):
                    t = hh.tile([128, MH], bf16, tag=f"ph_fm{c}", name=f"ph_fm{b}{h}{c}")
                    nc.vector.tensor_tensor(out=t[:], in0=fmr01[c][:, hsl], in1=mrep[:, hsl],
                                            op=ALU.mult)
                    ph_fm.append(t)
                w_eng = nc.gpsimd if GWGB else nc.vector
                w_g = hh.tile([128, MH], bf16, tag="w_g", name=f"w_g{b}{h}")
                w_eng.tensor_tensor(out=w_g[:], in0=old_rep[0][:, hsl], in1=mrep[:, hsl],
                                    op=ALU.mult)
                w_b = hh.tile([128, MH], bf16, tag="w_b", name=f"w_b{b}{h}")
                w_eng.tensor_tensor(out=w_b[:], in0=old_rep[1][:, hsl], in1=mrep[:, hsl],
                                    op=ALU.mult)
                PH[(b, h)] = dict(P2x=P2x, P3=P3, P4=P4, P5=P5, ph_fm=ph_fm, w_g=w_g, w_b=w_b)

            # ---------------- main tile (two stages, software-pipelined) ----------------
            def tile_stage1(b, h, nt):
                """matmuls + (-200*s) fp16 copy + max tree -> nm. Returns psum+e ctx."""
                t = TP[b]
                p = PH[(b, h)]
                stt = stats[b]
                nsl = slice(nt * 128, (nt + 1) * 128)
                col = nt * NH + h
                ps = pm.tile([128, MH], f32, tag="ps", name=f"ps{b}{h}{nt}")
                chunks = [
                    (t["T2x"][0:128, nsl], p["P2x"], 128),
                    (t["T3"][0:104, nsl], p["P3"], 104),
                    (t["T4"][0:104, nsl], p["P4"], 104),
                    (t["T5"][0:72, nsl], p["P5"], 72),
                    (t["th_fm"][0][:, nsl], p["ph_fm"][0], 128),
                    (t["th_fm"][1][:, nsl], p["ph_fm"][1], 128),
                ]
                nchunks = len(chunks)
                for ci, (lhsT, ph, rows) in enumerate(chunks):
                    for k in range(NSUB):
                        pcol = slice(k * 512, (k + 1) * 512)
                        nc.tensor.matmul(ps[:, pcol], lhsT, ph[0:rows, pcol],
                                         start=(ci == 0), stop=(ci == nchunks - 1))

                # nm = min(-200*s) = -200*max(s) via fp16 copy (ulp(13000)=8 logits
                # below overflow; exp arg stays within ~8 of 0: safe in fp32)
                s2 = wk.tile([128, MH], f16, tag="s2", name=f"s2_{b}{h}{nt}", bufs=2)
                nc.scalar.mul(s2[:], ps[:], -200.0)
                m1 = wk.tile([128, MH // 2], f16, tag="m1", name=f"m1_{b}{h}{nt}", bufs=2)
                nc.vector.tensor_tensor(out=m1[:], in0=s2[:, 0:MH // 2], in1=s2[:, MH // 2:MH],
                                        op=ALU.min)
                m2 = wk.tile([128, MH // 4], f16, tag="m2", name=f"m2_{b}{h}{nt}", bufs=1)
                nc.vector.tensor_tensor(out=m2[:], in0=m1[:, 0:MH // 4], in1=m1[:, MH // 4:MH // 2],
                                        op=ALU.min)
                m3 = wk.tile([128, MH // 8], f16, tag="m3", name=f"m3_{b}{h}{nt}", bufs=1)
                nc.vector.tensor_tensor(out=m3[:], in0=m2[:, 0:MH // 8], in1=m2[:, MH // 8:MH // 4],
                                        op=ALU.min)
                nc.vector.tensor_reduce(out=stt["nm"][:, col:col + 1], in_=m3[:],
                                        axis=mybir.AxisListType.X, op=ALU.min)
                return (b, h, nt, ps)

            def tile_stage2(ctx):
                """exp + weighted sums for a tile whose stage1 already ran."""
                b, h, nt, ps = ctx
                p = PH[(b, h)]
                stt = stats[b]
                col = nt * NH + h
                e_t = wk.tile([128, MH], bf16, tag="e", name=f"e{b}{h}{nt}", bufs=2)
                nc.scalar.activation(
                    out=e_t[:], in_=ps[:], func=AF.Exp,
                    bias=stt["nm"][:, col:col + 1], scale=200.0,
                    accum_out=stt["z"][:, col:col + 1],
                )
                for key, wvec in (("g", p["w_g"]), ("b", p["w_b"])):
                    if BYP:
                        nc.vector.scalar_tensor_tensor(
                            out=junk_d[:], in0=e_t[:], scalar=0.0,
                            in1=wvec[:], op0=ALU.bypass, op1=ALU.mult,
                            accum_out=stt[f"n{key}"][:, col:col + 1],
                        )
                    else:
                        nc.vector.scalar_tensor_tensor(
                            out=junk_d[:], in0=e_t[:], scalar=1.0,
                            in1=wvec[:], op0=ALU.mult, op1=ALU.mult,
                            accum_out=stt[f"n{key}"][:, col:col + 1],
                        )

            # ---------------- schedule ----------------
            steps = [(b, h) for b in range(NP) for h in range(NH)]
            part_prep(0)
            half_prep(0, 0)
            half_prep(0, 1)
            tiles = [(b, h, nt) for (b, h) in steps for nt in range(NT)]
            pending = None  # stage2 of the previous tile, emitted one tile late
            for ti, (b, h, nt) in enumerate(tiles):
                ctx = tile_stage1(b, h, nt)
                if pending is not None:
                    tile_stage2(pending)
                pending = ctx
                # emit prep two halves ahead, at the end of each (b,h) group
                if nt == NT - 1:
                    si = steps.index((b, h))
                    ni = si + 2
                    if ni < len(steps):
                        nb_, nh_ = steps[ni]
                        if nh_ == 0:
                            part_prep(nb_)
                        half_prep(nb_, nh_)
            tile_stage2(pending)

            # ---------------- epilogue: combine stats, assemble output ----------------
            gacc = st.tile([128, NP * NT], f32)
            bacc_t = st.tile([128, NP * NT], f32)
            for b in range(NP):
                stt = stats[b]
                nm2 = stt["nm"][:].rearrange("p (t h) -> p t h", h=NH)
                nmm = sp.tile([128, NT], f32, tag=f"nmm{b}", name=f"nmm{b}")
                nc.vector.tensor_reduce(out=nmm[:], in_=nm2, axis=mybir.AxisListType.X, op=ALU.min)
                d2 = sp.tile([128, NT, NH], f32, tag=f"d2{b}", name=f"d2{b}")
                for h in range(NH):
                    nc.vector.tensor_tensor(out=d2[:, :, h], in0=nmm[:], in1=nm2[:, :, h],
                                            op=ALU.subtract)
                c2 = sp.tile([128, NT, NH], f32, tag=f"c2{b}", name=f"c2{b}")
                nc.scalar.activation(out=c2[:], in_=d2[:], func=AF.Exp)
                for name, s1, acc in (("z", "z", None),
                                      ("g", "ng", gacc),
                                      ("bb", "nb", bacc_t)):
                    tot = stt[s1]
                    sc = sp.tile([128, NT, NH], f32, tag=f"sc_{name}{b}", name=f"sc_{name}{b}")
                    nc.vector.tensor_tensor(out=sc[:], in0=tot[:].rearrange("p (t h) -> p t h", h=NH),
                                            in1=c2[:], op=ALU.mult)
                    if name == "z":
                        zi = sp.tile([128, NT], f32, tag=f"zi{b}", name=f"zi{b}")
                        nc.vector.tensor_reduce(out=zi[:], in_=sc[:], axis=mybir.AxisListType.X,
                                                op=ALU.add)
                        rz = sp.tile([128, NT], f32, tag=f"rz{b}", name=f"rz{b}")
                        nc.vector.reciprocal(rz[:], zi[:])
                    else:
                        si_t = sp.tile([128, NT], f32, tag=f"si_{name}{b}", name=f"si_{name}{b}")
                        nc.vector.tensor_reduce(out=si_t[:], in_=sc[:], axis=mybir.AxisListType.X,
                                                op=ALU.add)
                        nc.vector.tensor_tensor(out=acc[:, b * NT:(b + 1) * NT], in0=si_t[:],
                                                in1=rz[:], op=ALU.mult)

            gb_sb = st.tile([128, 8], f32)
            nc.vector.tensor_reduce(out=gb_sb[:, 0:NT],
                                    in_=gacc[:].rearrange("p (b t) -> p t b", b=NP),
                                    axis=mybir.AxisListType.X, op=ALU.add)
            nc.vector.tensor_scalar_add(gb_sb[:, 0:NT], gb_sb[:, 0:NT], 1.0)
            nc.vector.tensor_reduce(out=gb_sb[:, NT:8],
                                    in_=bacc_t[:].rearrange("p (b t) -> p t b", b=NP),
                                    axis=mybir.AxisListType.X, op=ALU.add)

            ps_t = pm.tile([128, MH], f32, tag="ps", name="ps_tr")
            nc.tensor.transpose(ps_t[:8, 0:128], gb_sb[:], ident[:])
            gb_t = st.tile([8, 128], f32)
            nc.scalar.copy(gb_t[:], ps_t[:8, 0:128])
            nc.sync.dma_start(out=gb_dram[:], in_=gb_t[:])

            g1_rep = st.tile([128, S], f32)
            nc.sync.dma_start(out=g1_rep[:],
                              in_=gb_dram[0:NT, :].unsqueeze(0).to_broadcast([128, NT, 128]))
            b_rep = st.tile([128, S], f32)
            nc.sync.dma_start(out=b_rep[:],
                              in_=gb_dram[NT:8, :].unsqueeze(0).to_broadcast([128, NT, 128]))

            for c in range(2):
                o_t = wk.tile([128, S], f32, tag="o", name=f"o{c}", bufs=2)
                nc.vector.tensor_tensor(out=o_t[:], in0=fms_raw[c][:], in1=g1_rep[:], op=ALU.mult)
                nc.vector.tensor_tensor(out=o_t[:], in0=o_t[:], in1=b_rep[:], op=ALU.add)
                nc.sync.dma_start(out=out_ext[c * 128:(c + 1) * 128, :], in_=o_t[:])

    nc.compile()
    return nc


def kernel(fm_source, fm_reference, mask_source, mask_ref,
           rel_pos_source, rel_pos_ref, w_lambda, w_beta):
    from concourse.bass_utils import run_bass_kernel_spmd

    if "nc" not in _cache:
        _cache["nc"] = _build()
    nc = _cache["nc"]

    fm_src = np.ascontiguousarray(np.asarray(fm_source, np.float32).reshape(C_FM, HW))
    fm_refm = np.ascontiguousarray(np.asarray(fm_reference, np.float32).reshape(C_FM, HW))
    m_src = np.ascontiguousarray(np.asarray(mask_source, np.float32).reshape(NP, HW))
    m_ref = np.ascontiguousarray(np.asarray(mask_ref, np.float32).reshape(NP, HW))
    r_src = np.ascontiguousarray(np.asarray(rel_pos_source, np.float32).reshape(NP, C_REL, HW))
    r_ref = np.ascontiguousarray(np.asarray(rel_pos_ref, np.float32).reshape(NP, C_REL, HW))
    w_l = np.ascontiguousarray(np.asarray(w_lambda, np.float32).reshape(1, C_FM))
    w_b = np.ascontiguousarray(np.asarray(w_beta, np.float32).reshape(1, C_FM))

    in_maps = []
    for k in range(NCORES):
        sl = slice(k * S, (k + 1) * S)
        in_maps.append({
            "fm_src_sl": np.ascontiguousarray(fm_src[:, sl]),
            "mask_src_sl": np.ascontiguousarray(m_src[:, sl]),
            "rel_src_sl": np.ascontiguousarray(r_src[:, :, sl]),
            "fm_ref": fm_refm,
            "mask_ref": m_ref,
            "rel_ref": r_ref,
            "w_lambda": w_l,
            "w_beta": w_b,
        })

    res = run_bass_kernel_spmd(nc, in_maps, list(range(NCORES)))
    _cache["last_result"] = res

    out = np.concatenate([res.results[k]["out"] for k in range(NCORES)], axis=1)
    return out.reshape(1, C_FM, H, W).astype(np.float32)

